# revision 22
# baseline (speedup 1.0000x reference)
"""Trainium2 Bass kernel for nn_BlockPiecewiseLinear (histogram_binning).

Math reformulation (validated vs the JAX reference to ~6e-7 rel):
    S    = softplus(slope)                      # [.., K+1]
    xs   = sort(x_pos, axis=-1)                 # [.., K]
    dS_j = S[j] - S[j-1]            (j = 1..K, stored at 0..K-1)
    step_j = 1[xs[j-1] <= q]
    A    = sum_j step_j * dS_j
    W    = sum_j step_j * dS_j * xs[j-1]
    slope_sel = (S[0]+EPS) + A
    out  = q*slope_sel - xs[0]*(S[0]+EPS) + xs[0] - W + y_bias

Sharding: pure data-parallel over the batch dim across 8 NeuronCores.
Per-core layout: rows (b,f) flattened; each SBUF tile is [128 part, G rows, K knots].
The 32-knot sort is an alternating-direction bitonic network (15 layers, 50
tensor_tensor min/max ops on AP-regular index subsets); everything else is
elementwise + free-dim reduces on DVE, softplus on ScalarE.
"""

import numpy as np

import concourse.bass as bass
import concourse.bacc as bacc
import concourse.mybir as mybir
import concourse.tile as tile
from concourse.bass_utils import run_bass_kernel_spmd

F32 = mybir.dt.float32
Alu = mybir.AluOpType
Act = mybir.ActivationFunctionType
AxX = mybir.AxisListType.X

B, F, K = 4096, 512, 32
KP1 = K + 1
EPS = 1e-3
NCORES = 8
P = 128
G = 128  # rows per partition per tile; P*G rows/tile


def _bitonic_layers(n=32):
    layers = []
    k = 2
    while k <= n:
        j = k // 2
        while j >= 1:
            layers.append((k, j))
            j //= 2
        k *= 2
    return layers  # 15 layers for n=32


def _emit_sort_layer(nc, cur, dst, kk, jj, g):
    """Alternating-direction bitonic layer (block size kk, distance jj).

    For kk < K the ascending and descending halves are fused into one min op
    and one max op: with index bits i = c*2k + d*k + m*2j + e*j + r, the min
    result goes to position e=d and the max to e=1-d, which stays AP-regular
    (the d-level step becomes k +/- j) and the g level coalesces with c.
    """
    if kk < K:
        # walrus DVE operands are TENSOR3D (3 free dims after coalescing):
        # the asc/desc halves must stay separate ops (their fused output
        # pattern needs 4 levels).
        cs = 16 // kk
        ms = kk // (2 * jj)
        vs = cur[:, :, :].rearrange(
            "p g (c d m e r) -> p g c d m e r", c=cs, d=2, m=ms, e=2, r=jj
        )
        vd = dst[:, :, :].rearrange(
            "p g (c d m e r) -> p g c d m e r", c=cs, d=2, m=ms, e=2, r=jj
        )
        a_lo = vs[:, :, :, 0, :, 0, :]
        a_hi = vs[:, :, :, 0, :, 1, :]
        nc.vector.tensor_tensor(out=vd[:, :, :, 0, :, 0, :], in0=a_lo, in1=a_hi, op=Alu.min)
        nc.vector.tensor_tensor(out=vd[:, :, :, 0, :, 1, :], in0=a_lo, in1=a_hi, op=Alu.max)
        d_lo = vs[:, :, :, 1, :, 0, :]
        d_hi = vs[:, :, :, 1, :, 1, :]
        nc.vector.tensor_tensor(out=vd[:, :, :, 1, :, 0, :], in0=d_lo, in1=d_hi, op=Alu.max)
        nc.vector.tensor_tensor(out=vd[:, :, :, 1, :, 1, :], in0=d_lo, in1=d_hi, op=Alu.min)
    else:
        ms = kk // (2 * jj)
        vs = cur[:, :, :].rearrange(
            "p g (m e r) -> p g m e r", m=ms, e=2, r=jj
        )
        vd = dst[:, :, :].rearrange(
            "p g (m e r) -> p g m e r", m=ms, e=2, r=jj
        )
        a_lo = vs[:, :, :, 0, :]
        a_hi = vs[:, :, :, 1, :]
        nc.vector.tensor_tensor(out=vd[:, :, :, 0, :], in0=a_lo, in1=a_hi, op=Alu.min)
        nc.vector.tensor_tensor(out=vd[:, :, :, 1, :], in0=a_lo, in1=a_hi, op=Alu.max)


def build_nc(nloc, g=G):
    rows_per_tile = P * g
    ntiles = nloc // rows_per_tile
    assert ntiles * rows_per_tile == nloc

    nc = bacc.Bacc("TRN2", target_bir_lowering=False, debug=False)
    x_d = nc.declare_dram_parameter("x", [nloc, K], F32, isOutput=False)
    sl_d = nc.declare_dram_parameter("sl", [nloc, KP1], F32, isOutput=False)
    q_d = nc.declare_dram_parameter("q", [nloc], F32, isOutput=False)
    yb_d = nc.declare_dram_parameter("yb", [P, g], F32, isOutput=False)
    out_d = nc.declare_dram_parameter("out", [nloc], F32, isOutput=True)
    ss_d = nc.declare_dram_parameter("ssel", [nloc], F32, isOutput=True)

    xv = x_d[:, :].rearrange("(t p g) k -> t p g k", p=P, g=g)
    slv = sl_d[:, :].rearrange("(t p g) k -> t p g k", p=P, g=g)
    qv = q_d[:].rearrange("(t p g) -> t p g", p=P, g=g)
    outv = out_d[:].rearrange("(t p g) -> t p g", p=P, g=g)
    ssv = ss_d[:].rearrange("(t p g) -> t p g", p=P, g=g)

    layers = _bitonic_layers(K)

    with tile.TileContext(nc) as tc:
        with (
            tc.tile_pool(name="pyb", bufs=1) as pyb,
            tc.tile_pool(name="px", bufs=2) as px,
            tc.tile_pool(name="psort", bufs=2) as psort,
            tc.tile_pool(name="psl", bufs=1) as psl,
            tc.tile_pool(name="pS", bufs=2) as pS,
            tc.tile_pool(name="ptmp", bufs=3) as ptmp,
            tc.tile_pool(name="psm", bufs=4) as psm,
            tc.tile_pool(name="pq", bufs=4) as pq,
            tc.tile_pool(name="pout", bufs=4) as pout,
        ):
            yb_t = pyb.tile([P, g], F32, tag="yb")
            nc.scalar.dma_start(out=yb_t[:, :], in_=yb_d[:, :])

            # stores of tile t-1 are emitted after tile t's softplus, so on
            # the in-order ACT queue exp/ln(t) aren't stuck behind a store
            # that waits on tile t-1's DVE epilogue
            pending = None
            for t in range(ntiles):
                x_t = px.tile([P, g, K], F32, tag="x")
                nc.scalar.dma_start(out=x_t[:, :, :], in_=xv[t])
                sl_t = psl.tile([P, g, KP1], F32, tag="sl")
                nc.scalar.dma_start(out=sl_t[:, :, :], in_=slv[t])
                q_t = pq.tile([P, g], F32, tag="q")
                nc.scalar.dma_start(out=q_t[:, :], in_=qv[t])

                # softplus = ln(1 + exp(x)); exp in-place on the slope tile
                nc.scalar.activation(
                    out=sl_t[:, :, :], in_=sl_t[:, :, :], func=Act.Exp
                )
                S_t = pS.tile([P, g, KP1], F32, tag="S")
                nc.scalar.activation(
                    out=S_t[:, :, :], in_=sl_t[:, :, :], func=Act.Ln, bias=1.0
                )

                # dS early so the ACT pipeline stays decoupled
                dS_t = ptmp.tile([P, g, K], F32, tag="w3")
                nc.vector.tensor_tensor(
                    out=dS_t[:, :, :], in0=S_t[:, :, 1:KP1], in1=S_t[:, :, 0:K],
                    op=Alu.subtract,
                )
                sm = psm.tile([P, g, 8], F32, tag="sm")
                s0p = sm[:, :, 2]
                nc.vector.tensor_scalar_add(s0p, S_t[:, :, 0], EPS)

                if pending is not None:
                    po, pot, ps, pst = pending
                    nc.scalar.dma_start(out=po, in_=pot[:, :])
                    nc.scalar.dma_start(out=ps, in_=pst[:, :])
                    pending = None

                # ---- bitonic sort of the K knots (ascending) ----
                cur = x_t
                for kk, jj in layers:
                    dst = psort.tile([P, g, K], F32, tag="sort")
                    _emit_sort_layer(nc, cur, dst, kk, jj, g)
                    cur = dst
                xs_t = cur  # sorted ascending [P, g, K]

                # ---- knot-dim elementwise + reduces (DVE) ----
                step_t = ptmp.tile([P, g, K], F32, tag="w3")
                xs_full = xs_t[:, :, :]
                q2ap = q_t[:, :]
                qb = bass.AP(
                    tensor=q2ap.tensor,
                    offset=q2ap.offset,
                    ap=[q2ap.ap[0], q2ap.ap[1], [0, K]],
                )
                nc.vector.tensor_tensor(
                    out=step_t[:, :, :], in0=xs_full, in1=qb, op=Alu.is_le
                )
                m_t = ptmp.tile([P, g, K], F32, tag="w3")
                nc.vector.tensor_tensor(
                    out=m_t[:, :, :], in0=dS_t[:, :, :], in1=step_t[:, :, :],
                    op=Alu.mult,
                )
                w_t = ptmp.tile([P, g, K], F32, tag="w3")
                nc.vector.tensor_tensor(
                    out=w_t[:, :, :], in0=m_t[:, :, :], in1=xs_full, op=Alu.mult
                )

                A = sm[:, :, 0]
                W = sm[:, :, 1]
                nc.vector.tensor_reduce(out=A, in_=m_t[:, :, :], axis=AxX, op=Alu.add)
                nc.vector.tensor_reduce(out=W, in_=w_t[:, :, :], axis=AxX, op=Alu.add)

                # ---- epilogue on [P, g] (DVE):
                #   ssel = s0p + A
                #   out  = q*ssel - xmin*s0p + xmin - W + yb
                q2 = q_t[:, :]
                xmin = xs_t[:, :, 0]
                u = sm[:, :, 4]
                v = sm[:, :, 5]
                r = sm[:, :, 6]
                out_t = pout.tile([P, g], F32, tag="out")
                ss_t = pout.tile([P, g], F32, tag="ss")
                nc.vector.tensor_tensor(out=ss_t[:, :], in0=s0p, in1=A, op=Alu.add)
                nc.vector.tensor_tensor(out=u, in0=q2, in1=ss_t[:, :], op=Alu.mult)
                nc.vector.tensor_tensor(out=v, in0=xmin, in1=s0p, op=Alu.mult)
                nc.vector.tensor_tensor(out=r, in0=u, in1=v, op=Alu.subtract)
                nc.vector.tensor_tensor(out=u, in0=r, in1=xmin, op=Alu.add)
                nc.vector.tensor_tensor(out=v, in0=u, in1=W, op=Alu.subtract)
                nc.vector.tensor_tensor(out=out_t[:, :], in0=v, in1=yb_t[:, :], op=Alu.add)

                pending = (outv[t], out_t, ssv[t], ss_t)
            po, pot, ps, pst = pending
            nc.scalar.dma_start(out=po, in_=pot[:, :])
            nc.scalar.dma_start(out=ps, in_=pst[:, :])
    nc.compile()
    return nc


_NC_CACHE = {}


def _get_nc(nloc, g=G):
    key = (nloc, g)
    if key not in _NC_CACHE:
        _NC_CACHE[key] = build_nc(nloc, g)
    return _NC_CACHE[key]


def kernel(inputs, x_pos, slope, y_bias):
    inputs = np.ascontiguousarray(np.asarray(inputs, dtype=np.float32))
    x_pos = np.ascontiguousarray(np.asarray(x_pos, dtype=np.float32))
    slope = np.ascontiguousarray(np.asarray(slope, dtype=np.float32))
    y_bias = np.ascontiguousarray(np.asarray(y_bias, dtype=np.float32))

    b, f = inputs.shape
    bloc = b // NCORES
    nloc = bloc * f
    nc = _get_nc(nloc)

    # y_bias expanded to the [P, G] per-tile layout: row (p, g) has f = (p*G+g) % F
    yb_exp = np.ascontiguousarray(np.tile(y_bias[:, 0], (P * G) // f).reshape(P, G))

    in_maps = []
    for c in range(NCORES):
        sl_b = slice(c * bloc, (c + 1) * bloc)
        in_maps.append(
            {
                "x": x_pos[sl_b].reshape(nloc, K),
                "sl": slope[sl_b].reshape(nloc, KP1),
                "q": inputs[sl_b].reshape(nloc),
                "yb": yb_exp,
            }
        )

    res = run_bass_kernel_spmd(nc, in_maps, list(range(NCORES)))
    outs = np.concatenate(
        [res.results[c]["out"].reshape(bloc, f) for c in range(NCORES)], axis=0
    )
    ssel = np.concatenate(
        [res.results[c]["ssel"].reshape(bloc, f) for c in range(NCORES)], axis=0
    )
    return outs, ssel


# revision 23
# speedup vs baseline: 1.0003x; 1.0003x over previous
"""Trainium2 Bass kernel for nn_BlockPiecewiseLinear (histogram_binning).

Math reformulation (validated vs the JAX reference to ~6e-7 rel):
    S    = softplus(slope)                      # [.., K+1]
    xs   = sort(x_pos, axis=-1)                 # [.., K]
    dS_j = S[j] - S[j-1]            (j = 1..K, stored at 0..K-1)
    step_j = 1[xs[j-1] <= q]
    A    = sum_j step_j * dS_j
    W    = sum_j step_j * dS_j * xs[j-1]
    slope_sel = (S[0]+EPS) + A
    out  = q*slope_sel - xs[0]*(S[0]+EPS) + xs[0] - W + y_bias

Sharding: pure data-parallel over the batch dim across 8 NeuronCores.
Per-core layout: rows (b,f) flattened; each SBUF tile is [128 part, G rows, K knots].
The 32-knot sort is an alternating-direction bitonic network (15 layers, 50
tensor_tensor min/max ops on AP-regular index subsets); everything else is
elementwise + free-dim reduces on DVE, softplus on ScalarE.
"""

import numpy as np

import concourse.bass as bass
import concourse.bacc as bacc
import concourse.mybir as mybir
import concourse.tile as tile
from concourse.bass_utils import run_bass_kernel_spmd

F32 = mybir.dt.float32
Alu = mybir.AluOpType
Act = mybir.ActivationFunctionType
AxX = mybir.AxisListType.X

B, F, K = 4096, 512, 32
KP1 = K + 1
EPS = 1e-3
NCORES = 8
P = 128
G = 128  # rows per partition per tile; P*G rows/tile


def _bitonic_layers(n=32):
    layers = []
    k = 2
    while k <= n:
        j = k // 2
        while j >= 1:
            layers.append((k, j))
            j //= 2
        k *= 2
    return layers  # 15 layers for n=32


def _emit_sort_layer(nc, cur, dst, kk, jj, g):
    """Alternating-direction bitonic layer (block size kk, distance jj).

    For kk < K the ascending and descending halves are fused into one min op
    and one max op: with index bits i = c*2k + d*k + m*2j + e*j + r, the min
    result goes to position e=d and the max to e=1-d, which stays AP-regular
    (the d-level step becomes k +/- j) and the g level coalesces with c.
    """
    if kk < K:
        # walrus DVE operands are TENSOR3D (3 free dims after coalescing):
        # the asc/desc halves must stay separate ops (their fused output
        # pattern needs 4 levels).
        cs = 16 // kk
        ms = kk // (2 * jj)
        vs = cur[:, :, :].rearrange(
            "p g (c d m e r) -> p g c d m e r", c=cs, d=2, m=ms, e=2, r=jj
        )
        vd = dst[:, :, :].rearrange(
            "p g (c d m e r) -> p g c d m e r", c=cs, d=2, m=ms, e=2, r=jj
        )
        a_lo = vs[:, :, :, 0, :, 0, :]
        a_hi = vs[:, :, :, 0, :, 1, :]
        nc.vector.tensor_tensor(out=vd[:, :, :, 0, :, 0, :], in0=a_lo, in1=a_hi, op=Alu.min)
        nc.vector.tensor_tensor(out=vd[:, :, :, 0, :, 1, :], in0=a_lo, in1=a_hi, op=Alu.max)
        d_lo = vs[:, :, :, 1, :, 0, :]
        d_hi = vs[:, :, :, 1, :, 1, :]
        nc.vector.tensor_tensor(out=vd[:, :, :, 1, :, 0, :], in0=d_lo, in1=d_hi, op=Alu.max)
        nc.vector.tensor_tensor(out=vd[:, :, :, 1, :, 1, :], in0=d_lo, in1=d_hi, op=Alu.min)
    else:
        ms = kk // (2 * jj)
        vs = cur[:, :, :].rearrange(
            "p g (m e r) -> p g m e r", m=ms, e=2, r=jj
        )
        vd = dst[:, :, :].rearrange(
            "p g (m e r) -> p g m e r", m=ms, e=2, r=jj
        )
        a_lo = vs[:, :, :, 0, :]
        a_hi = vs[:, :, :, 1, :]
        nc.vector.tensor_tensor(out=vd[:, :, :, 0, :], in0=a_lo, in1=a_hi, op=Alu.min)
        nc.vector.tensor_tensor(out=vd[:, :, :, 1, :], in0=a_lo, in1=a_hi, op=Alu.max)


def build_nc(nloc, g=G):
    rows_per_tile = P * g
    ntiles = nloc // rows_per_tile
    assert ntiles * rows_per_tile == nloc

    nc = bacc.Bacc("TRN2", target_bir_lowering=False, debug=False)
    x_d = nc.declare_dram_parameter("x", [nloc, K], F32, isOutput=False)
    sl_d = nc.declare_dram_parameter("sl", [nloc, KP1], F32, isOutput=False)
    q_d = nc.declare_dram_parameter("q", [nloc], F32, isOutput=False)
    yb_d = nc.declare_dram_parameter("yb", [P, g], F32, isOutput=False)
    out_d = nc.declare_dram_parameter("out", [nloc], F32, isOutput=True)
    ss_d = nc.declare_dram_parameter("ssel", [nloc], F32, isOutput=True)

    xv = x_d[:, :].rearrange("(t p g) k -> t p g k", p=P, g=g)
    slv = sl_d[:, :].rearrange("(t p g) k -> t p g k", p=P, g=g)
    qv = q_d[:].rearrange("(t p g) -> t p g", p=P, g=g)
    outv = out_d[:].rearrange("(t p g) -> t p g", p=P, g=g)
    ssv = ss_d[:].rearrange("(t p g) -> t p g", p=P, g=g)

    layers = _bitonic_layers(K)

    with tile.TileContext(nc) as tc:
        with (
            tc.tile_pool(name="pyb", bufs=1) as pyb,
            tc.tile_pool(name="px", bufs=2) as px,
            tc.tile_pool(name="psort", bufs=2) as psort,
            tc.tile_pool(name="psl", bufs=1) as psl,
            tc.tile_pool(name="pS", bufs=2) as pS,
            tc.tile_pool(name="ptmp", bufs=3) as ptmp,
            tc.tile_pool(name="psm", bufs=4) as psm,
            tc.tile_pool(name="pq", bufs=4) as pq,
            tc.tile_pool(name="pout", bufs=4) as pout,
        ):
            yb_t = pyb.tile([P, g], F32, tag="yb")
            nc.scalar.dma_start(out=yb_t[:, :], in_=yb_d[:, :])

            # stores of tile t-1 are emitted after tile t's softplus, so on
            # the in-order ACT queue exp/ln(t) aren't stuck behind a store
            # that waits on tile t-1's DVE epilogue
            fin = None
            for t in range(ntiles):
                x_t = px.tile([P, g, K], F32, tag="x")
                nc.scalar.dma_start(out=x_t[:, :, :], in_=xv[t])
                sl_t = psl.tile([P, g, KP1], F32, tag="sl")
                nc.scalar.dma_start(out=sl_t[:, :, :], in_=slv[t])
                q_t = pq.tile([P, g], F32, tag="q")
                nc.scalar.dma_start(out=q_t[:, :], in_=qv[t])

                # softplus = ln(1 + exp(x)); exp in-place on the slope tile
                nc.scalar.activation(
                    out=sl_t[:, :, :], in_=sl_t[:, :, :], func=Act.Exp
                )
                S_t = pS.tile([P, g, KP1], F32, tag="S")
                nc.scalar.activation(
                    out=S_t[:, :, :], in_=sl_t[:, :, :], func=Act.Ln, bias=1.0
                )

                # dS early so the ACT pipeline stays decoupled
                dS_t = ptmp.tile([P, g, K], F32, tag="w3")
                nc.vector.tensor_tensor(
                    out=dS_t[:, :, :], in0=S_t[:, :, 1:KP1], in1=S_t[:, :, 0:K],
                    op=Alu.subtract,
                )
                sm = psm.tile([P, g, 8], F32, tag="sm")
                s0p = sm[:, :, 2]
                nc.vector.tensor_scalar_add(s0p, S_t[:, :, 0], EPS)

                if fin is not None:
                    po_u, po_W, po_v, po_ot, po_st, po_o, po_s = fin
                    nc.vector.tensor_tensor(out=po_v, in0=po_u, in1=po_W, op=Alu.subtract)
                    nc.vector.tensor_tensor(out=po_ot[:, :], in0=po_v, in1=yb_t[:, :], op=Alu.add)
                    nc.scalar.dma_start(out=po_o, in_=po_ot[:, :])
                    nc.scalar.dma_start(out=po_s, in_=po_st[:, :])
                    fin = None

                # ---- bitonic sort of the K knots (ascending) ----
                cur = x_t
                for kk, jj in layers:
                    dst = psort.tile([P, g, K], F32, tag="sort")
                    _emit_sort_layer(nc, cur, dst, kk, jj, g)
                    cur = dst
                xs_t = cur  # sorted ascending [P, g, K]

                # ---- knot-dim elementwise + reduces (DVE) ----
                step_t = ptmp.tile([P, g, K], F32, tag="w3")
                xs_full = xs_t[:, :, :]
                q2ap = q_t[:, :]
                qb = bass.AP(
                    tensor=q2ap.tensor,
                    offset=q2ap.offset,
                    ap=[q2ap.ap[0], q2ap.ap[1], [0, K]],
                )
                nc.vector.tensor_tensor(
                    out=step_t[:, :, :], in0=xs_full, in1=qb, op=Alu.is_le
                )
                m_t = ptmp.tile([P, g, K], F32, tag="w3")
                nc.vector.tensor_tensor(
                    out=m_t[:, :, :], in0=dS_t[:, :, :], in1=step_t[:, :, :],
                    op=Alu.mult,
                )
                w_t = ptmp.tile([P, g, K], F32, tag="w3")
                nc.vector.tensor_tensor(
                    out=w_t[:, :, :], in0=m_t[:, :, :], in1=xs_full, op=Alu.mult
                )

                # epilogue ops that don't need W go between the w-mult and
                # the W-reduce so the DVE pipe drain of w is hidden
                A = sm[:, :, 0]
                W = sm[:, :, 1]
                nc.vector.tensor_reduce(out=A, in_=m_t[:, :, :], axis=AxX, op=Alu.add)

                # ---- epilogue on [P, g] (DVE):
                #   ssel = s0p + A
                #   out  = q*ssel - xmin*s0p + xmin - W + yb
                q2 = q_t[:, :]
                xmin = xs_t[:, :, 0]
                u = sm[:, :, 4]
                v = sm[:, :, 5]
                r = sm[:, :, 6]
                out_t = pout.tile([P, g], F32, tag="out")
                ss_t = pout.tile([P, g], F32, tag="ss")
                nc.vector.tensor_tensor(out=ss_t[:, :], in0=s0p, in1=A, op=Alu.add)
                nc.vector.tensor_tensor(out=u, in0=q2, in1=ss_t[:, :], op=Alu.mult)
                nc.vector.tensor_tensor(out=v, in0=xmin, in1=s0p, op=Alu.mult)
                nc.vector.tensor_tensor(out=r, in0=u, in1=v, op=Alu.subtract)
                nc.vector.tensor_tensor(out=u, in0=r, in1=xmin, op=Alu.add)
                nc.vector.tensor_reduce(out=W, in_=w_t[:, :, :], axis=AxX, op=Alu.add)
                # the two W-dependent ops are deferred into the next
                # iteration (after its dS/s0p ops) to hide W's pipe drain
                fin = (u, W, v, out_t, ss_t, outv[t], ssv[t])

            po_u, po_W, po_v, po_ot, po_st, po_o, po_s = fin
            nc.vector.tensor_tensor(out=po_v, in0=po_u, in1=po_W, op=Alu.subtract)
            nc.vector.tensor_tensor(out=po_ot[:, :], in0=po_v, in1=yb_t[:, :], op=Alu.add)
            nc.scalar.dma_start(out=po_o, in_=po_ot[:, :])
            nc.scalar.dma_start(out=po_s, in_=po_st[:, :])
    nc.compile()
    return nc


_NC_CACHE = {}


def _get_nc(nloc, g=G):
    key = (nloc, g)
    if key not in _NC_CACHE:
        _NC_CACHE[key] = build_nc(nloc, g)
    return _NC_CACHE[key]


def kernel(inputs, x_pos, slope, y_bias):
    inputs = np.ascontiguousarray(np.asarray(inputs, dtype=np.float32))
    x_pos = np.ascontiguousarray(np.asarray(x_pos, dtype=np.float32))
    slope = np.ascontiguousarray(np.asarray(slope, dtype=np.float32))
    y_bias = np.ascontiguousarray(np.asarray(y_bias, dtype=np.float32))

    b, f = inputs.shape
    bloc = b // NCORES
    nloc = bloc * f
    nc = _get_nc(nloc)

    # y_bias expanded to the [P, G] per-tile layout: row (p, g) has f = (p*G+g) % F
    yb_exp = np.ascontiguousarray(np.tile(y_bias[:, 0], (P * G) // f).reshape(P, G))

    in_maps = []
    for c in range(NCORES):
        sl_b = slice(c * bloc, (c + 1) * bloc)
        in_maps.append(
            {
                "x": x_pos[sl_b].reshape(nloc, K),
                "sl": slope[sl_b].reshape(nloc, KP1),
                "q": inputs[sl_b].reshape(nloc),
                "yb": yb_exp,
            }
        )

    res = run_bass_kernel_spmd(nc, in_maps, list(range(NCORES)))
    outs = np.concatenate(
        [res.results[c]["out"].reshape(bloc, f) for c in range(NCORES)], axis=0
    )
    ssel = np.concatenate(
        [res.results[c]["ssel"].reshape(bloc, f) for c in range(NCORES)], axis=0
    )
    return outs, ssel


# revision 24
# speedup vs baseline: 1.0013x; 1.0009x over previous
"""Trainium2 Bass kernel for nn_BlockPiecewiseLinear (histogram_binning).

Math reformulation (validated vs the JAX reference to ~6e-7 rel):
    S    = softplus(slope)                      # [.., K+1]
    xs   = sort(x_pos, axis=-1)                 # [.., K]
    dS_j = S[j] - S[j-1]            (j = 1..K, stored at 0..K-1)
    step_j = 1[xs[j-1] <= q]
    A    = sum_j step_j * dS_j
    W    = sum_j step_j * dS_j * xs[j-1]
    slope_sel = (S[0]+EPS) + A
    out  = q*slope_sel - xs[0]*(S[0]+EPS) + xs[0] - W + y_bias

Sharding: pure data-parallel over the batch dim across 8 NeuronCores.
Per-core layout: rows (b,f) flattened; each SBUF tile is [128 part, G rows, K knots].
The 32-knot sort is an alternating-direction bitonic network (15 layers, 50
tensor_tensor min/max ops on AP-regular index subsets); everything else is
elementwise + free-dim reduces on DVE, softplus on ScalarE.
"""

import numpy as np

import concourse.bass as bass
import concourse.bacc as bacc
import concourse.mybir as mybir
import concourse.tile as tile
from concourse.bass_utils import run_bass_kernel_spmd

F32 = mybir.dt.float32
Alu = mybir.AluOpType
Act = mybir.ActivationFunctionType
AxX = mybir.AxisListType.X

B, F, K = 4096, 512, 32
KP1 = K + 1
EPS = 1e-3
NCORES = 8
P = 128
G = 128  # rows per partition per tile; P*G rows/tile


def _bitonic_layers(n=32):
    layers = []
    k = 2
    while k <= n:
        j = k // 2
        while j >= 1:
            layers.append((k, j))
            j //= 2
        k *= 2
    return layers  # 15 layers for n=32


def _emit_sort_layer(nc, cur, dst, kk, jj, g):
    """Alternating-direction bitonic layer (block size kk, distance jj).

    Index bits i = c*2k + d*k + m*2j + e*j + r; d selects sort direction.
    walrus lowers DVE operands as TENSOR3D (3 free dims after stride
    coalescing), so the ascending and descending halves must stay separate
    min/max ops: their fused output pattern would need 4 levels.
    """
    if kk < K:
        cs = 16 // kk
        ms = kk // (2 * jj)
        vs = cur[:, :, :].rearrange(
            "p g (c d m e r) -> p g c d m e r", c=cs, d=2, m=ms, e=2, r=jj
        )
        vd = dst[:, :, :].rearrange(
            "p g (c d m e r) -> p g c d m e r", c=cs, d=2, m=ms, e=2, r=jj
        )
        a_lo = vs[:, :, :, 0, :, 0, :]
        a_hi = vs[:, :, :, 0, :, 1, :]
        nc.vector.tensor_tensor(out=vd[:, :, :, 0, :, 0, :], in0=a_lo, in1=a_hi, op=Alu.min)
        nc.vector.tensor_tensor(out=vd[:, :, :, 0, :, 1, :], in0=a_lo, in1=a_hi, op=Alu.max)
        d_lo = vs[:, :, :, 1, :, 0, :]
        d_hi = vs[:, :, :, 1, :, 1, :]
        nc.vector.tensor_tensor(out=vd[:, :, :, 1, :, 0, :], in0=d_lo, in1=d_hi, op=Alu.max)
        nc.vector.tensor_tensor(out=vd[:, :, :, 1, :, 1, :], in0=d_lo, in1=d_hi, op=Alu.min)
    else:
        ms = kk // (2 * jj)
        vs = cur[:, :, :].rearrange(
            "p g (m e r) -> p g m e r", m=ms, e=2, r=jj
        )
        vd = dst[:, :, :].rearrange(
            "p g (m e r) -> p g m e r", m=ms, e=2, r=jj
        )
        a_lo = vs[:, :, :, 0, :]
        a_hi = vs[:, :, :, 1, :]
        nc.vector.tensor_tensor(out=vd[:, :, :, 0, :], in0=a_lo, in1=a_hi, op=Alu.min)
        nc.vector.tensor_tensor(out=vd[:, :, :, 1, :], in0=a_lo, in1=a_hi, op=Alu.max)


def build_nc(nloc, g=G):
    rows_per_tile = P * g
    ntiles = nloc // rows_per_tile
    assert ntiles * rows_per_tile == nloc

    nc = bacc.Bacc("TRN2", target_bir_lowering=False, debug=False)
    x_d = nc.declare_dram_parameter("x", [nloc, K], F32, isOutput=False)
    sl_d = nc.declare_dram_parameter("sl", [nloc, KP1], F32, isOutput=False)
    q_d = nc.declare_dram_parameter("q", [nloc], F32, isOutput=False)
    yb_d = nc.declare_dram_parameter("yb", [P, g], F32, isOutput=False)
    out_d = nc.declare_dram_parameter("out", [nloc], F32, isOutput=True)
    ss_d = nc.declare_dram_parameter("ssel", [nloc], F32, isOutput=True)

    xv = x_d[:, :].rearrange("(t p g) k -> t p g k", p=P, g=g)
    slv = sl_d[:, :].rearrange("(t p g) k -> t p g k", p=P, g=g)
    qv = q_d[:].rearrange("(t p g) -> t p g", p=P, g=g)
    outv = out_d[:].rearrange("(t p g) -> t p g", p=P, g=g)
    ssv = ss_d[:].rearrange("(t p g) -> t p g", p=P, g=g)

    layers = _bitonic_layers(K)

    with tile.TileContext(nc) as tc:
        with (
            tc.tile_pool(name="pyb", bufs=1) as pyb,
            tc.tile_pool(name="px", bufs=2) as px,
            tc.tile_pool(name="psort", bufs=2) as psort,
            tc.tile_pool(name="psl", bufs=1) as psl,
            tc.tile_pool(name="pS", bufs=2) as pS,
            tc.tile_pool(name="ptmp", bufs=3) as ptmp,
            tc.tile_pool(name="psm", bufs=4) as psm,
            tc.tile_pool(name="pq", bufs=4) as pq,
            tc.tile_pool(name="pout", bufs=4) as pout,
        ):
            yb_t = pyb.tile([P, g], F32, tag="yb")
            nc.scalar.dma_start(out=yb_t[:, :], in_=yb_d[:, :])

            # stores of tile t-1 are emitted after tile t's softplus, so on
            # the in-order ACT queue exp/ln(t) aren't stuck behind a store
            # that waits on tile t-1's DVE epilogue
            fin = None
            for t in range(ntiles):
                x_t = px.tile([P, g, K], F32, tag="x")
                nc.scalar.dma_start(out=x_t[:, :, :], in_=xv[t])
                sl_t = psl.tile([P, g, KP1], F32, tag="sl")
                nc.scalar.dma_start(out=sl_t[:, :, :], in_=slv[t])
                q_t = pq.tile([P, g], F32, tag="q")
                nc.scalar.dma_start(out=q_t[:, :], in_=qv[t])

                # softplus = ln(1 + exp(x)); exp in-place on the slope tile
                nc.scalar.activation(
                    out=sl_t[:, :, :], in_=sl_t[:, :, :], func=Act.Exp
                )
                S_t = pS.tile([P, g, KP1], F32, tag="S")
                nc.scalar.activation(
                    out=S_t[:, :, :], in_=sl_t[:, :, :], func=Act.Ln, bias=1.0
                )

                # dS early so the ACT pipeline stays decoupled
                dS_t = ptmp.tile([P, g, K], F32, tag="w3")
                nc.vector.tensor_tensor(
                    out=dS_t[:, :, :], in0=S_t[:, :, 1:KP1], in1=S_t[:, :, 0:K],
                    op=Alu.subtract,
                )
                sm = psm.tile([P, g, 8], F32, tag="sm")
                s0p = sm[:, :, 2]
                nc.vector.tensor_scalar_add(s0p, S_t[:, :, 0], EPS)

                if fin is not None:
                    po_u, po_W, po_v, po_ot, po_st, po_o, po_s = fin
                    nc.vector.tensor_tensor(out=po_v, in0=po_u, in1=po_W, op=Alu.subtract)
                    nc.vector.tensor_tensor(out=po_ot[:, :], in0=po_v, in1=yb_t[:, :], op=Alu.add)
                    nc.scalar.dma_start(out=po_o, in_=po_ot[:, :])
                    nc.scalar.dma_start(out=po_s, in_=po_st[:, :])
                    fin = None

                # ---- bitonic sort of the K knots (ascending) ----
                cur = x_t
                for kk, jj in layers:
                    dst = psort.tile([P, g, K], F32, tag="sort")
                    _emit_sort_layer(nc, cur, dst, kk, jj, g)
                    cur = dst
                xs_t = cur  # sorted ascending [P, g, K]

                # ---- knot-dim elementwise + reduces (DVE) ----
                step_t = ptmp.tile([P, g, K], F32, tag="w3")
                xs_full = xs_t[:, :, :]
                q2ap = q_t[:, :]
                qb = bass.AP(
                    tensor=q2ap.tensor,
                    offset=q2ap.offset,
                    ap=[q2ap.ap[0], q2ap.ap[1], [0, K]],
                )
                nc.vector.tensor_tensor(
                    out=step_t[:, :, :], in0=xs_full, in1=qb, op=Alu.is_le
                )
                m_t = ptmp.tile([P, g, K], F32, tag="w3")
                nc.vector.tensor_tensor(
                    out=m_t[:, :, :], in0=dS_t[:, :, :], in1=step_t[:, :, :],
                    op=Alu.mult,
                )
                w_t = ptmp.tile([P, g, K], F32, tag="w3")
                nc.vector.tensor_tensor(
                    out=w_t[:, :, :], in0=m_t[:, :, :], in1=xs_full, op=Alu.mult
                )

                # epilogue ops that don't need W go between the w-mult and
                # the W-reduce so the DVE pipe drain of w is hidden
                A = sm[:, :, 0]
                W = sm[:, :, 1]
                nc.vector.tensor_reduce(out=A, in_=m_t[:, :, :], axis=AxX, op=Alu.add)

                # ---- epilogue on [P, g] (DVE):
                #   ssel = s0p + A
                #   out  = q*ssel - xmin*s0p + xmin - W + yb
                q2 = q_t[:, :]
                xmin = xs_t[:, :, 0]
                u = sm[:, :, 4]
                v = sm[:, :, 5]
                r = sm[:, :, 6]
                out_t = pout.tile([P, g], F32, tag="out")
                ss_t = pout.tile([P, g], F32, tag="ss")
                nc.vector.tensor_tensor(out=ss_t[:, :], in0=s0p, in1=A, op=Alu.add)
                nc.vector.tensor_tensor(out=u, in0=q2, in1=ss_t[:, :], op=Alu.mult)
                nc.vector.tensor_tensor(out=v, in0=xmin, in1=s0p, op=Alu.mult)
                nc.vector.tensor_tensor(out=r, in0=u, in1=v, op=Alu.subtract)
                nc.vector.tensor_tensor(out=u, in0=r, in1=xmin, op=Alu.add)
                nc.vector.tensor_reduce(out=W, in_=w_t[:, :, :], axis=AxX, op=Alu.add)
                # the two W-dependent ops are deferred into the next
                # iteration (after its dS/s0p ops) to hide W's pipe drain
                fin = (u, W, v, out_t, ss_t, outv[t], ssv[t])

            po_u, po_W, po_v, po_ot, po_st, po_o, po_s = fin
            nc.vector.tensor_tensor(out=po_v, in0=po_u, in1=po_W, op=Alu.subtract)
            nc.vector.tensor_tensor(out=po_ot[:, :], in0=po_v, in1=yb_t[:, :], op=Alu.add)
            nc.scalar.dma_start(out=po_o, in_=po_ot[:, :])
            nc.scalar.dma_start(out=po_s, in_=po_st[:, :])
    nc.compile()
    return nc


_NC_CACHE = {}


def _get_nc(nloc, g=G):
    key = (nloc, g)
    if key not in _NC_CACHE:
        _NC_CACHE[key] = build_nc(nloc, g)
    return _NC_CACHE[key]


def kernel(inputs, x_pos, slope, y_bias):
    inputs = np.ascontiguousarray(np.asarray(inputs, dtype=np.float32))
    x_pos = np.ascontiguousarray(np.asarray(x_pos, dtype=np.float32))
    slope = np.ascontiguousarray(np.asarray(slope, dtype=np.float32))
    y_bias = np.ascontiguousarray(np.asarray(y_bias, dtype=np.float32))

    b, f = inputs.shape
    bloc = b // NCORES
    nloc = bloc * f
    nc = _get_nc(nloc)

    # y_bias expanded to the [P, G] per-tile layout: row (p, g) has f = (p*G+g) % F
    yb_exp = np.ascontiguousarray(np.tile(y_bias[:, 0], (P * G) // f).reshape(P, G))

    in_maps = []
    for c in range(NCORES):
        sl_b = slice(c * bloc, (c + 1) * bloc)
        in_maps.append(
            {
                "x": x_pos[sl_b].reshape(nloc, K),
                "sl": slope[sl_b].reshape(nloc, KP1),
                "q": inputs[sl_b].reshape(nloc),
                "yb": yb_exp,
            }
        )

    res = run_bass_kernel_spmd(nc, in_maps, list(range(NCORES)))
    outs = np.concatenate(
        [res.results[c]["out"].reshape(bloc, f) for c in range(NCORES)], axis=0
    )
    ssel = np.concatenate(
        [res.results[c]["ssel"].reshape(bloc, f) for c in range(NCORES)], axis=0
    )
    return outs, ssel


# revision 25
# speedup vs baseline: 1.0023x; 1.0010x over previous
"""Trainium2 Bass kernel for nn_BlockPiecewiseLinear (histogram_binning).

Math reformulation (validated vs the JAX reference to ~6e-7 rel):
    S    = softplus(slope)                      # [.., K+1]
    xs   = sort(x_pos, axis=-1)                 # [.., K]
    dS_j = S[j] - S[j-1]            (j = 1..K, stored at 0..K-1)
    step_j = 1[xs[j-1] <= q]
    A    = sum_j step_j * dS_j
    W    = sum_j step_j * dS_j * xs[j-1]
    slope_sel = (S[0]+EPS) + A
    out  = q*slope_sel - xs[0]*(S[0]+EPS) + xs[0] - W + y_bias

Sharding: pure data-parallel over the batch dim across 8 NeuronCores.
Per-core layout: rows (b,f) flattened; each SBUF tile is [128 part, G rows, K knots].
The 32-knot sort is an alternating-direction bitonic network (15 layers, 50
tensor_tensor min/max ops on AP-regular index subsets); everything else is
elementwise + free-dim reduces on DVE, softplus on ScalarE.
"""

import numpy as np

import concourse.bass as bass
import concourse.bacc as bacc
import concourse.mybir as mybir
import concourse.tile as tile
from concourse.bass_utils import run_bass_kernel_spmd

F32 = mybir.dt.float32
Alu = mybir.AluOpType
Act = mybir.ActivationFunctionType
AxX = mybir.AxisListType.X

B, F, K = 4096, 512, 32
KP1 = K + 1
EPS = 1e-3
NCORES = 8
P = 128
G = 128  # rows per partition per tile; P*G rows/tile


def _bitonic_layers(n=32):
    layers = []
    k = 2
    while k <= n:
        j = k // 2
        while j >= 1:
            layers.append((k, j))
            j //= 2
        k *= 2
    return layers  # 15 layers for n=32


def _emit_sort_layer(nc, cur, dst, kk, jj, g):
    """Alternating-direction bitonic layer (block size kk, distance jj).

    Index bits i = c*2k + d*k + m*2j + e*j + r; d selects sort direction.
    walrus lowers DVE operands as TENSOR3D (3 free dims after stride
    coalescing), so the ascending and descending halves must stay separate
    min/max ops: their fused output pattern would need 4 levels.
    """
    if kk == 16 and jj == 8:
        # special case: c and m dims collapse, so asc+desc fuse into one
        # min op and one max op within TENSOR3D's 3-free-dim limit.
        # i = d*16 + e*8 + r; min -> d*24 + r, max -> 8 + d*8 + r
        base_s = cur[:, :, :]
        base_d = dst[:, :, :]
        in_lo = bass.AP(tensor=base_s.tensor, offset=base_s.offset,
                        ap=[base_s.ap[0], [32, g], [16, 2], [1, 8]])
        in_hi = bass.AP(tensor=base_s.tensor, offset=base_s.offset + 8,
                        ap=[base_s.ap[0], [32, g], [16, 2], [1, 8]])
        out_min = bass.AP(tensor=base_d.tensor, offset=base_d.offset,
                          ap=[base_d.ap[0], [32, g], [24, 2], [1, 8]])
        out_max = bass.AP(tensor=base_d.tensor, offset=base_d.offset + 8,
                          ap=[base_d.ap[0], [32, g], [8, 2], [1, 8]])
        nc.vector.tensor_tensor(out=out_min, in0=in_lo, in1=in_hi, op=Alu.min)
        nc.vector.tensor_tensor(out=out_max, in0=in_lo, in1=in_hi, op=Alu.max)
    elif kk < K:
        cs = 16 // kk
        ms = kk // (2 * jj)
        vs = cur[:, :, :].rearrange(
            "p g (c d m e r) -> p g c d m e r", c=cs, d=2, m=ms, e=2, r=jj
        )
        vd = dst[:, :, :].rearrange(
            "p g (c d m e r) -> p g c d m e r", c=cs, d=2, m=ms, e=2, r=jj
        )
        a_lo = vs[:, :, :, 0, :, 0, :]
        a_hi = vs[:, :, :, 0, :, 1, :]
        nc.vector.tensor_tensor(out=vd[:, :, :, 0, :, 0, :], in0=a_lo, in1=a_hi, op=Alu.min)
        nc.vector.tensor_tensor(out=vd[:, :, :, 0, :, 1, :], in0=a_lo, in1=a_hi, op=Alu.max)
        d_lo = vs[:, :, :, 1, :, 0, :]
        d_hi = vs[:, :, :, 1, :, 1, :]
        nc.vector.tensor_tensor(out=vd[:, :, :, 1, :, 0, :], in0=d_lo, in1=d_hi, op=Alu.max)
        nc.vector.tensor_tensor(out=vd[:, :, :, 1, :, 1, :], in0=d_lo, in1=d_hi, op=Alu.min)
    else:
        ms = kk // (2 * jj)
        vs = cur[:, :, :].rearrange(
            "p g (m e r) -> p g m e r", m=ms, e=2, r=jj
        )
        vd = dst[:, :, :].rearrange(
            "p g (m e r) -> p g m e r", m=ms, e=2, r=jj
        )
        a_lo = vs[:, :, :, 0, :]
        a_hi = vs[:, :, :, 1, :]
        nc.vector.tensor_tensor(out=vd[:, :, :, 0, :], in0=a_lo, in1=a_hi, op=Alu.min)
        nc.vector.tensor_tensor(out=vd[:, :, :, 1, :], in0=a_lo, in1=a_hi, op=Alu.max)


def build_nc(nloc, g=G):
    rows_per_tile = P * g
    ntiles = nloc // rows_per_tile
    assert ntiles * rows_per_tile == nloc

    nc = bacc.Bacc("TRN2", target_bir_lowering=False, debug=False)
    x_d = nc.declare_dram_parameter("x", [nloc, K], F32, isOutput=False)
    sl_d = nc.declare_dram_parameter("sl", [nloc, KP1], F32, isOutput=False)
    q_d = nc.declare_dram_parameter("q", [nloc], F32, isOutput=False)
    yb_d = nc.declare_dram_parameter("yb", [P, g], F32, isOutput=False)
    out_d = nc.declare_dram_parameter("out", [nloc], F32, isOutput=True)
    ss_d = nc.declare_dram_parameter("ssel", [nloc], F32, isOutput=True)

    xv = x_d[:, :].rearrange("(t p g) k -> t p g k", p=P, g=g)
    slv = sl_d[:, :].rearrange("(t p g) k -> t p g k", p=P, g=g)
    qv = q_d[:].rearrange("(t p g) -> t p g", p=P, g=g)
    outv = out_d[:].rearrange("(t p g) -> t p g", p=P, g=g)
    ssv = ss_d[:].rearrange("(t p g) -> t p g", p=P, g=g)

    layers = _bitonic_layers(K)

    with tile.TileContext(nc) as tc:
        with (
            tc.tile_pool(name="pyb", bufs=1) as pyb,
            tc.tile_pool(name="px", bufs=2) as px,
            tc.tile_pool(name="psort", bufs=2) as psort,
            tc.tile_pool(name="psl", bufs=1) as psl,
            tc.tile_pool(name="pS", bufs=2) as pS,
            tc.tile_pool(name="ptmp", bufs=3) as ptmp,
            tc.tile_pool(name="psm", bufs=4) as psm,
            tc.tile_pool(name="pq", bufs=4) as pq,
            tc.tile_pool(name="pout", bufs=4) as pout,
        ):
            yb_t = pyb.tile([P, g], F32, tag="yb")
            nc.scalar.dma_start(out=yb_t[:, :], in_=yb_d[:, :])

            # stores of tile t-1 are emitted after tile t's softplus, so on
            # the in-order ACT queue exp/ln(t) aren't stuck behind a store
            # that waits on tile t-1's DVE epilogue
            fin = None
            for t in range(ntiles):
                x_t = px.tile([P, g, K], F32, tag="x")
                nc.scalar.dma_start(out=x_t[:, :, :], in_=xv[t])
                sl_t = psl.tile([P, g, KP1], F32, tag="sl")
                nc.scalar.dma_start(out=sl_t[:, :, :], in_=slv[t])
                q_t = pq.tile([P, g], F32, tag="q")
                nc.scalar.dma_start(out=q_t[:, :], in_=qv[t])

                # softplus = ln(1 + exp(x)); exp in-place on the slope tile
                nc.scalar.activation(
                    out=sl_t[:, :, :], in_=sl_t[:, :, :], func=Act.Exp
                )
                S_t = pS.tile([P, g, KP1], F32, tag="S")
                nc.scalar.activation(
                    out=S_t[:, :, :], in_=sl_t[:, :, :], func=Act.Ln, bias=1.0
                )

                # dS early so the ACT pipeline stays decoupled
                dS_t = ptmp.tile([P, g, K], F32, tag="w3")
                nc.vector.tensor_tensor(
                    out=dS_t[:, :, :], in0=S_t[:, :, 1:KP1], in1=S_t[:, :, 0:K],
                    op=Alu.subtract,
                )
                sm = psm.tile([P, g, 8], F32, tag="sm")
                s0p = sm[:, :, 2]
                nc.vector.tensor_scalar_add(s0p, S_t[:, :, 0], EPS)

                if fin is not None:
                    po_u, po_W, po_v, po_ot, po_st, po_o, po_s = fin
                    nc.vector.tensor_tensor(out=po_v, in0=po_u, in1=po_W, op=Alu.subtract)
                    nc.vector.tensor_tensor(out=po_ot[:, :], in0=po_v, in1=yb_t[:, :], op=Alu.add)
                    nc.scalar.dma_start(out=po_o, in_=po_ot[:, :])
                    nc.scalar.dma_start(out=po_s, in_=po_st[:, :])
                    fin = None

                # ---- bitonic sort of the K knots (ascending) ----
                cur = x_t
                for kk, jj in layers:
                    dst = psort.tile([P, g, K], F32, tag="sort")
                    _emit_sort_layer(nc, cur, dst, kk, jj, g)
                    cur = dst
                xs_t = cur  # sorted ascending [P, g, K]

                # ---- knot-dim elementwise + reduces (DVE) ----
                step_t = ptmp.tile([P, g, K], F32, tag="w3")
                xs_full = xs_t[:, :, :]
                q2ap = q_t[:, :]
                qb = bass.AP(
                    tensor=q2ap.tensor,
                    offset=q2ap.offset,
                    ap=[q2ap.ap[0], q2ap.ap[1], [0, K]],
                )
                nc.vector.tensor_tensor(
                    out=step_t[:, :, :], in0=xs_full, in1=qb, op=Alu.is_le
                )
                m_t = ptmp.tile([P, g, K], F32, tag="w3")
                nc.vector.tensor_tensor(
                    out=m_t[:, :, :], in0=dS_t[:, :, :], in1=step_t[:, :, :],
                    op=Alu.mult,
                )
                w_t = ptmp.tile([P, g, K], F32, tag="w3")
                nc.vector.tensor_tensor(
                    out=w_t[:, :, :], in0=m_t[:, :, :], in1=xs_full, op=Alu.mult
                )

                # epilogue ops that don't need W go between the w-mult and
                # the W-reduce so the DVE pipe drain of w is hidden
                A = sm[:, :, 0]
                W = sm[:, :, 1]
                nc.vector.tensor_reduce(out=A, in_=m_t[:, :, :], axis=AxX, op=Alu.add)

                # ---- epilogue on [P, g] (DVE):
                #   ssel = s0p + A
                #   out  = q*ssel - xmin*s0p + xmin - W + yb
                q2 = q_t[:, :]
                xmin = xs_t[:, :, 0]
                u = sm[:, :, 4]
                v = sm[:, :, 5]
                r = sm[:, :, 6]
                out_t = pout.tile([P, g], F32, tag="out")
                ss_t = pout.tile([P, g], F32, tag="ss")
                nc.vector.tensor_tensor(out=ss_t[:, :], in0=s0p, in1=A, op=Alu.add)
                nc.vector.tensor_tensor(out=u, in0=q2, in1=ss_t[:, :], op=Alu.mult)
                nc.vector.tensor_tensor(out=v, in0=xmin, in1=s0p, op=Alu.mult)
                nc.vector.tensor_tensor(out=r, in0=u, in1=v, op=Alu.subtract)
                nc.vector.tensor_tensor(out=u, in0=r, in1=xmin, op=Alu.add)
                nc.vector.tensor_reduce(out=W, in_=w_t[:, :, :], axis=AxX, op=Alu.add)
                # the two W-dependent ops are deferred into the next
                # iteration (after its dS/s0p ops) to hide W's pipe drain
                fin = (u, W, v, out_t, ss_t, outv[t], ssv[t])

            po_u, po_W, po_v, po_ot, po_st, po_o, po_s = fin
            nc.vector.tensor_tensor(out=po_v, in0=po_u, in1=po_W, op=Alu.subtract)
            nc.vector.tensor_tensor(out=po_ot[:, :], in0=po_v, in1=yb_t[:, :], op=Alu.add)
            nc.scalar.dma_start(out=po_o, in_=po_ot[:, :])
            nc.scalar.dma_start(out=po_s, in_=po_st[:, :])
    nc.compile()
    return nc


_NC_CACHE = {}


def _get_nc(nloc, g=G):
    key = (nloc, g)
    if key not in _NC_CACHE:
        _NC_CACHE[key] = build_nc(nloc, g)
    return _NC_CACHE[key]


def kernel(inputs, x_pos, slope, y_bias):
    inputs = np.ascontiguousarray(np.asarray(inputs, dtype=np.float32))
    x_pos = np.ascontiguousarray(np.asarray(x_pos, dtype=np.float32))
    slope = np.ascontiguousarray(np.asarray(slope, dtype=np.float32))
    y_bias = np.ascontiguousarray(np.asarray(y_bias, dtype=np.float32))

    b, f = inputs.shape
    bloc = b // NCORES
    nloc = bloc * f
    nc = _get_nc(nloc)

    # y_bias expanded to the [P, G] per-tile layout: row (p, g) has f = (p*G+g) % F
    yb_exp = np.ascontiguousarray(np.tile(y_bias[:, 0], (P * G) // f).reshape(P, G))

    in_maps = []
    for c in range(NCORES):
        sl_b = slice(c * bloc, (c + 1) * bloc)
        in_maps.append(
            {
                "x": x_pos[sl_b].reshape(nloc, K),
                "sl": slope[sl_b].reshape(nloc, KP1),
                "q": inputs[sl_b].reshape(nloc),
                "yb": yb_exp,
            }
        )

    res = run_bass_kernel_spmd(nc, in_maps, list(range(NCORES)))
    outs = np.concatenate(
        [res.results[c]["out"].reshape(bloc, f) for c in range(NCORES)], axis=0
    )
    ssel = np.concatenate(
        [res.results[c]["ssel"].reshape(bloc, f) for c in range(NCORES)], axis=0
    )
    return outs, ssel


# revision 26
# speedup vs baseline: 1.0346x; 1.0322x over previous
"""Trainium2 Bass kernel for nn_BlockPiecewiseLinear (histogram_binning).

Math reformulation (validated vs the JAX reference to ~6e-7 rel):
    S    = softplus(slope)                      # [.., K+1]
    xs   = sort(x_pos, axis=-1)                 # [.., K]
    dS_j = S[j] - S[j-1]            (j = 1..K, stored at 0..K-1)
    step_j = 1[xs[j-1] <= q]
    A    = sum_j step_j * dS_j
    W    = sum_j step_j * dS_j * xs[j-1]
    slope_sel = (S[0]+EPS) + A
    out  = q*slope_sel - xs[0]*(S[0]+EPS) + xs[0] - W + y_bias

Sharding: pure data-parallel over the batch dim across 8 NeuronCores.
Per-core layout: rows (b,f) flattened; each SBUF tile is [128 part, G rows, K knots].
The 32-knot sort is an alternating-direction bitonic network (15 layers, 50
tensor_tensor min/max ops on AP-regular index subsets); everything else is
elementwise + free-dim reduces on DVE, softplus on ScalarE.
"""

import numpy as np

import concourse.bass as bass
import concourse.bacc as bacc
import concourse.mybir as mybir
import concourse.tile as tile
from concourse.bass_utils import run_bass_kernel_spmd

F32 = mybir.dt.float32
Alu = mybir.AluOpType
Act = mybir.ActivationFunctionType
AxX = mybir.AxisListType.X

B, F, K = 4096, 512, 32
KP1 = K + 1
EPS = 1e-3
NCORES = 8
P = 128
G = 128  # rows per partition per tile; P*G rows/tile


def _bitonic_layers(n=32):
    layers = []
    k = 2
    while k <= n:
        j = k // 2
        while j >= 1:
            layers.append((k, j))
            j //= 2
        k *= 2
    return layers  # 15 layers for n=32


def _emit_sort_layer(nc, cur, dst, kk, jj, g):
    """Alternating-direction bitonic layer (block size kk, distance jj).

    Index bits i = c*2k + d*k + m*2j + e*j + r; d selects sort direction.
    walrus lowers DVE operands as TENSOR3D (3 free dims after stride
    coalescing), so the ascending and descending halves must stay separate
    min/max ops: their fused output pattern would need 4 levels.
    """
    if kk < K and jj == kk // 2:
        # first sub-layer of each k-stage: the m dim is unit and g always
        # coalesces with c (32 = 2k * 16/k), so asc+desc fuse into one min
        # and one max op within TENSOR3D's 3-free-dim limit.
        # i = c*2k + d*k + e*j + r; min -> c*2k + d*(k+j) + r,
        # max -> j + c*2k + d*(k-j) + r
        cs = 16 // kk
        base_s = cur[:, :, :]
        base_d = dst[:, :, :]
        rlev = ([[1, jj]] if jj > 1 else [])
        in_ap = [base_s.ap[0], [2 * kk, g * cs], [kk, 2]] + rlev
        in_lo = bass.AP(tensor=base_s.tensor, offset=base_s.offset, ap=in_ap)
        in_hi = bass.AP(tensor=base_s.tensor, offset=base_s.offset + jj, ap=in_ap)
        out_min = bass.AP(tensor=base_d.tensor, offset=base_d.offset,
                          ap=[base_d.ap[0], [2 * kk, g * cs], [kk + jj, 2]] + rlev)
        out_max = bass.AP(tensor=base_d.tensor, offset=base_d.offset + jj,
                          ap=[base_d.ap[0], [2 * kk, g * cs], [kk - jj, 2]] + rlev)
        nc.vector.tensor_tensor(out=out_min, in0=in_lo, in1=in_hi, op=Alu.min)
        nc.vector.tensor_tensor(out=out_max, in0=in_lo, in1=in_hi, op=Alu.max)
    elif kk < K:
        cs = 16 // kk
        ms = kk // (2 * jj)
        vs = cur[:, :, :].rearrange(
            "p g (c d m e r) -> p g c d m e r", c=cs, d=2, m=ms, e=2, r=jj
        )
        vd = dst[:, :, :].rearrange(
            "p g (c d m e r) -> p g c d m e r", c=cs, d=2, m=ms, e=2, r=jj
        )
        a_lo = vs[:, :, :, 0, :, 0, :]
        a_hi = vs[:, :, :, 0, :, 1, :]
        nc.vector.tensor_tensor(out=vd[:, :, :, 0, :, 0, :], in0=a_lo, in1=a_hi, op=Alu.min)
        nc.vector.tensor_tensor(out=vd[:, :, :, 0, :, 1, :], in0=a_lo, in1=a_hi, op=Alu.max)
        d_lo = vs[:, :, :, 1, :, 0, :]
        d_hi = vs[:, :, :, 1, :, 1, :]
        nc.vector.tensor_tensor(out=vd[:, :, :, 1, :, 0, :], in0=d_lo, in1=d_hi, op=Alu.max)
        nc.vector.tensor_tensor(out=vd[:, :, :, 1, :, 1, :], in0=d_lo, in1=d_hi, op=Alu.min)
    else:
        ms = kk // (2 * jj)
        vs = cur[:, :, :].rearrange(
            "p g (m e r) -> p g m e r", m=ms, e=2, r=jj
        )
        vd = dst[:, :, :].rearrange(
            "p g (m e r) -> p g m e r", m=ms, e=2, r=jj
        )
        a_lo = vs[:, :, :, 0, :]
        a_hi = vs[:, :, :, 1, :]
        nc.vector.tensor_tensor(out=vd[:, :, :, 0, :], in0=a_lo, in1=a_hi, op=Alu.min)
        nc.vector.tensor_tensor(out=vd[:, :, :, 1, :], in0=a_lo, in1=a_hi, op=Alu.max)


def build_nc(nloc, g=G):
    rows_per_tile = P * g
    ntiles = nloc // rows_per_tile
    assert ntiles * rows_per_tile == nloc

    nc = bacc.Bacc("TRN2", target_bir_lowering=False, debug=False)
    x_d = nc.declare_dram_parameter("x", [nloc, K], F32, isOutput=False)
    sl_d = nc.declare_dram_parameter("sl", [nloc, KP1], F32, isOutput=False)
    q_d = nc.declare_dram_parameter("q", [nloc], F32, isOutput=False)
    yb_d = nc.declare_dram_parameter("yb", [P, g], F32, isOutput=False)
    out_d = nc.declare_dram_parameter("out", [nloc], F32, isOutput=True)
    ss_d = nc.declare_dram_parameter("ssel", [nloc], F32, isOutput=True)

    xv = x_d[:, :].rearrange("(t p g) k -> t p g k", p=P, g=g)
    slv = sl_d[:, :].rearrange("(t p g) k -> t p g k", p=P, g=g)
    qv = q_d[:].rearrange("(t p g) -> t p g", p=P, g=g)
    outv = out_d[:].rearrange("(t p g) -> t p g", p=P, g=g)
    ssv = ss_d[:].rearrange("(t p g) -> t p g", p=P, g=g)

    layers = _bitonic_layers(K)

    with tile.TileContext(nc) as tc:
        with (
            tc.tile_pool(name="pyb", bufs=1) as pyb,
            tc.tile_pool(name="px", bufs=2) as px,
            tc.tile_pool(name="psort", bufs=2) as psort,
            tc.tile_pool(name="psl", bufs=1) as psl,
            tc.tile_pool(name="pS", bufs=2) as pS,
            tc.tile_pool(name="ptmp", bufs=3) as ptmp,
            tc.tile_pool(name="psm", bufs=4) as psm,
            tc.tile_pool(name="pq", bufs=4) as pq,
            tc.tile_pool(name="pout", bufs=4) as pout,
        ):
            yb_t = pyb.tile([P, g], F32, tag="yb")
            nc.scalar.dma_start(out=yb_t[:, :], in_=yb_d[:, :])

            # stores of tile t-1 are emitted after tile t's softplus, so on
            # the in-order ACT queue exp/ln(t) aren't stuck behind a store
            # that waits on tile t-1's DVE epilogue
            fin = None
            for t in range(ntiles):
                x_t = px.tile([P, g, K], F32, tag="x")
                nc.scalar.dma_start(out=x_t[:, :, :], in_=xv[t])
                sl_t = psl.tile([P, g, KP1], F32, tag="sl")
                nc.scalar.dma_start(out=sl_t[:, :, :], in_=slv[t])
                q_t = pq.tile([P, g], F32, tag="q")
                nc.scalar.dma_start(out=q_t[:, :], in_=qv[t])

                # softplus = ln(1 + exp(x)); exp in-place on the slope tile
                nc.scalar.activation(
                    out=sl_t[:, :, :], in_=sl_t[:, :, :], func=Act.Exp
                )
                S_t = pS.tile([P, g, KP1], F32, tag="S")
                nc.scalar.activation(
                    out=S_t[:, :, :], in_=sl_t[:, :, :], func=Act.Ln, bias=1.0
                )

                # dS early so the ACT pipeline stays decoupled
                dS_t = ptmp.tile([P, g, K], F32, tag="w3")
                nc.vector.tensor_tensor(
                    out=dS_t[:, :, :], in0=S_t[:, :, 1:KP1], in1=S_t[:, :, 0:K],
                    op=Alu.subtract,
                )
                sm = psm.tile([P, g, 8], F32, tag="sm")
                s0p = sm[:, :, 2]
                nc.vector.tensor_scalar_add(s0p, S_t[:, :, 0], EPS)

                if fin is not None:
                    po_u, po_W, po_v, po_ot, po_st, po_o, po_s = fin
                    nc.vector.tensor_tensor(out=po_v, in0=po_u, in1=po_W, op=Alu.subtract)
                    nc.vector.tensor_tensor(out=po_ot[:, :], in0=po_v, in1=yb_t[:, :], op=Alu.add)
                    nc.scalar.dma_start(out=po_o, in_=po_ot[:, :])
                    nc.scalar.dma_start(out=po_s, in_=po_st[:, :])
                    fin = None

                # ---- bitonic sort of the K knots (ascending) ----
                cur = x_t
                for kk, jj in layers:
                    dst = psort.tile([P, g, K], F32, tag="sort")
                    _emit_sort_layer(nc, cur, dst, kk, jj, g)
                    cur = dst
                xs_t = cur  # sorted ascending [P, g, K]

                # ---- knot-dim elementwise + reduces (DVE) ----
                step_t = ptmp.tile([P, g, K], F32, tag="w3")
                xs_full = xs_t[:, :, :]
                q2ap = q_t[:, :]
                qb = bass.AP(
                    tensor=q2ap.tensor,
                    offset=q2ap.offset,
                    ap=[q2ap.ap[0], q2ap.ap[1], [0, K]],
                )
                nc.vector.tensor_tensor(
                    out=step_t[:, :, :], in0=xs_full, in1=qb, op=Alu.is_le
                )
                m_t = ptmp.tile([P, g, K], F32, tag="w3")
                nc.vector.tensor_tensor(
                    out=m_t[:, :, :], in0=dS_t[:, :, :], in1=step_t[:, :, :],
                    op=Alu.mult,
                )
                w_t = ptmp.tile([P, g, K], F32, tag="w3")
                nc.vector.tensor_tensor(
                    out=w_t[:, :, :], in0=m_t[:, :, :], in1=xs_full, op=Alu.mult
                )

                # epilogue ops that don't need W go between the w-mult and
                # the W-reduce so the DVE pipe drain of w is hidden
                A = sm[:, :, 0]
                W = sm[:, :, 1]
                nc.vector.tensor_reduce(out=A, in_=m_t[:, :, :], axis=AxX, op=Alu.add)

                # ---- epilogue on [P, g] (DVE):
                #   ssel = s0p + A
                #   out  = q*ssel - xmin*s0p + xmin - W + yb
                q2 = q_t[:, :]
                xmin = xs_t[:, :, 0]
                u = sm[:, :, 4]
                v = sm[:, :, 5]
                r = sm[:, :, 6]
                out_t = pout.tile([P, g], F32, tag="out")
                ss_t = pout.tile([P, g], F32, tag="ss")
                nc.vector.tensor_tensor(out=ss_t[:, :], in0=s0p, in1=A, op=Alu.add)
                nc.vector.tensor_tensor(out=u, in0=q2, in1=ss_t[:, :], op=Alu.mult)
                nc.vector.tensor_tensor(out=v, in0=xmin, in1=s0p, op=Alu.mult)
                nc.vector.tensor_tensor(out=r, in0=u, in1=v, op=Alu.subtract)
                nc.vector.tensor_tensor(out=u, in0=r, in1=xmin, op=Alu.add)
                nc.vector.tensor_reduce(out=W, in_=w_t[:, :, :], axis=AxX, op=Alu.add)
                # the two W-dependent ops are deferred into the next
                # iteration (after its dS/s0p ops) to hide W's pipe drain
                fin = (u, W, v, out_t, ss_t, outv[t], ssv[t])

            po_u, po_W, po_v, po_ot, po_st, po_o, po_s = fin
            nc.vector.tensor_tensor(out=po_v, in0=po_u, in1=po_W, op=Alu.subtract)
            nc.vector.tensor_tensor(out=po_ot[:, :], in0=po_v, in1=yb_t[:, :], op=Alu.add)
            nc.scalar.dma_start(out=po_o, in_=po_ot[:, :])
            nc.scalar.dma_start(out=po_s, in_=po_st[:, :])
    nc.compile()
    return nc


_NC_CACHE = {}


def _get_nc(nloc, g=G):
    key = (nloc, g)
    if key not in _NC_CACHE:
        _NC_CACHE[key] = build_nc(nloc, g)
    return _NC_CACHE[key]


def kernel(inputs, x_pos, slope, y_bias):
    inputs = np.ascontiguousarray(np.asarray(inputs, dtype=np.float32))
    x_pos = np.ascontiguousarray(np.asarray(x_pos, dtype=np.float32))
    slope = np.ascontiguousarray(np.asarray(slope, dtype=np.float32))
    y_bias = np.ascontiguousarray(np.asarray(y_bias, dtype=np.float32))

    b, f = inputs.shape
    bloc = b // NCORES
    nloc = bloc * f
    nc = _get_nc(nloc)

    # y_bias expanded to the [P, G] per-tile layout: row (p, g) has f = (p*G+g) % F
    yb_exp = np.ascontiguousarray(np.tile(y_bias[:, 0], (P * G) // f).reshape(P, G))

    in_maps = []
    for c in range(NCORES):
        sl_b = slice(c * bloc, (c + 1) * bloc)
        in_maps.append(
            {
                "x": x_pos[sl_b].reshape(nloc, K),
                "sl": slope[sl_b].reshape(nloc, KP1),
                "q": inputs[sl_b].reshape(nloc),
                "yb": yb_exp,
            }
        )

    res = run_bass_kernel_spmd(nc, in_maps, list(range(NCORES)))
    outs = np.concatenate(
        [res.results[c]["out"].reshape(bloc, f) for c in range(NCORES)], axis=0
    )
    ssel = np.concatenate(
        [res.results[c]["ssel"].reshape(bloc, f) for c in range(NCORES)], axis=0
    )
    return outs, ssel
